# revision 3
# baseline (speedup 1.0000x reference)
"""KNNGraph v5 — v4 + interleaved kv tiles (fast start) + pb-first issue order.

Per core, per group of 128 query rows (32 groups):
  PE:  16 fp32r matmuls -> 8 PSUM blocks (128, 1024) of w = q.k - |k|^2/2
  Exit+pool per block-pair d (keys [1024d,1024d+1024) with +4096):
    'Y': ACT converts pb -> fp16 SBUF; DVE fused max(pa PSUM fp32, wb fp16)
    'X' (every X_DUTY-th group, pair 0): ACT converts both; DVE fp16 max
  DMA: ship p1 (128, 4096) fp16 per group.
Host: top-T pooled per row, expand x2 (keys j, j+4096), exact fp32 re-rank;
near-tie suspect rows -> exact full-row fallback.

kv is host-permuted into 4 tiles of [128, 2048]: tile d = keys
[1024d:1024d+1024) ++ [4096+1024d : 4096+1024d+1024), so each block-pair
depends on one 1MB DMA. q is split into 4 tiles of 8 groups each.
"""

import numpy as np

N, M, D = 4, 8192, 64
K = 16
NCORES = 8
QROWS = M // 2
NGROUPS = QROWS // 128  # 32
BLK = 1024
NBLK = 4
KDIM = 128
POOLW = M // 2          # 4096
EXPAND = 2
T_CAND = 24
DELTA = 0.25
X_DUTY = 0

_COMPILED = {}


def _is_x_pair(g, d, x_duty):
    # ~12 ACT-heavy pairs total balance DVE vs ACT busy time; placed mid-group
    # (d=2) and denser near (not at) the end so both engines drain together
    tail = NGROUPS - g
    if 3 <= tail <= 8 and d == 2:
        return True
    return bool(x_duty) and g % x_duty == x_duty - 1 and d == 2


def _build_nc(x_duty=None):
    import concourse.bacc as bacc
    import concourse.mybir as mybir
    import concourse.tile as tile

    x_duty = x_duty if x_duty is not None else X_DUTY

    nc = bacc.Bacc(None)
    f32 = mybir.dt.float32
    f32r = mybir.dt.float32r
    f16 = mybir.dt.float16

    q_d = nc.declare_dram_parameter("q", [KDIM, QROWS], f32r, isOutput=False)
    kv_d = nc.declare_dram_parameter("kv", [KDIM, M], f32r, isOutput=False)
    p1_d = nc.declare_dram_parameter("p1", [NGROUPS, 128, POOLW], f16, isOutput=True)

    with tile.TileContext(nc) as tc:
        with (
            tc.tile_pool(name="singles", bufs=1) as singles,
            tc.tile_pool(name="psum", bufs=2, space="PSUM") as psum,
            tc.tile_pool(name="w16", bufs=8) as w16pool,
            tc.tile_pool(name="pool1", bufs=6) as p1pool,
        ):
            NQ = 8
            q_sb = [singles.tile([KDIM, QROWS // NQ], f32r, name=f"q_sb{i}")
                    for i in range(NQ)]
            # kv tile d split into pb part (cols BLK..2BLK, needed first) and pa
            kv_sb = [singles.tile([KDIM, 2 * BLK], f32r, name=f"kv_sb{i}")
                     for i in range(NBLK)]
            # critical-path order on the hw queue: q tile 0, kv0 pb-half, kv0 pa-half
            nc.sync.dma_start(out=q_sb[0][:], in_=q_d[:, 0:QROWS // NQ])
            nc.sync.dma_start(out=kv_sb[0][:, BLK:2 * BLK], in_=kv_d[:, BLK:2 * BLK])
            nc.sync.dma_start(out=kv_sb[0][:, 0:BLK], in_=kv_d[:, 0:BLK])
            for d in range(1, NBLK):
                nc.sync.dma_start(out=kv_sb[d][:],
                                  in_=kv_d[:, 2 * BLK * d:2 * BLK * (d + 1)])
            for i in range(1, NQ):
                nc.sync.dma_start(
                    out=q_sb[i][:], in_=q_d[:, i * (QROWS // NQ):(i + 1) * (QROWS // NQ)])

            GPQ = NGROUPS // NQ  # groups per q tile
            for g in range(NGROUPS):
                qi, qr = g // GPQ, g % GPQ
                lhsT = q_sb[qi][:, qr * 128:(qr + 1) * 128]
                p1 = p1pool.tile([128, POOLW], f16, tag="p1")
                for d in range(NBLK):
                    pa = psum.tile([128, BLK], f32, tag="pa")
                    pb = psum.tile([128, BLK], f32, tag="pb")
                    for half in range(2):
                        nc.tensor.matmul(
                            pb[:, half * 512:(half + 1) * 512], lhsT,
                            kv_sb[d][:, BLK + half * 512:BLK + (half + 1) * 512],
                            start=True, stop=True)
                    wb = w16pool.tile([128, BLK], f16, tag="wb")
                    nc.scalar.copy(out=wb[:], in_=pb[:])
                    for half in range(2):
                        nc.tensor.matmul(
                            pa[:, half * 512:(half + 1) * 512], lhsT,
                            kv_sb[d][:, half * 512:(half + 1) * 512],
                            start=True, stop=True)
                    out_sl = p1[:, d * BLK:(d + 1) * BLK]
                    if _is_x_pair(g, d, x_duty):
                        wa = w16pool.tile([128, BLK], f16, tag="wa")
                        nc.scalar.copy(out=wa[:], in_=pa[:])
                        nc.vector.tensor_max(out_sl, wa[:], wb[:])
                    else:
                        nc.vector.tensor_max(out_sl, pa[:], wb[:])
                nc.sync.dma_start(out=p1_d[g, :, 0:POOLW // 2], in_=p1[:, 0:POOLW // 2])
                nc.sync.dma_start(out=p1_d[g, :, POOLW // 2:POOLW], in_=p1[:, POOLW // 2:POOLW])
    if not nc.is_finalized():
        nc.finalize()
    return nc


def _prep_inputs(x):
    x64 = x.astype(np.float64)
    x2 = (x64 * x64).sum(-1)
    neg_half_x2 = (-0.5 * x2).astype(np.float32)
    in_maps = []
    for c in range(NCORES):
        b, h = c // 2, c % 2
        q = np.zeros((KDIM, QROWS), np.float32)
        q[:D] = x[b, h * QROWS:(h + 1) * QROWS, :].T
        q[D] = 1.0
        kv = np.zeros((KDIM, M), np.float32)
        kv[:D] = x[b].T
        kv[D] = neg_half_x2[b]
        # permute into 4 tiles: tile d = cols [1024d:1024d+1024) ++ [4096+1024d:...)
        kvp = np.empty_like(kv)
        for d in range(NBLK):
            kvp[:, 2048 * d:2048 * d + 1024] = kv[:, 1024 * d:1024 * (d + 1)]
            kvp[:, 2048 * d + 1024:2048 * (d + 1)] = kv[:, POOLW + 1024 * d:POOLW + 1024 * (d + 1)]
        in_maps.append({"q": q, "kv": kvp})
    return in_maps


def _rerank_batch(xb, p1b):
    x2f = (xb * xb).sum(-1)
    p = p1b.astype(np.float32)
    np.nan_to_num(p, copy=False, nan=-np.inf)
    part = np.argpartition(-p, T_CAND - 1, axis=1)[:, :T_CAND]
    pv = np.take_along_axis(p, part, axis=1)
    pvs = -np.sort(-pv, axis=1)
    suspect = pvs[:, T_CAND - 1] >= pvs[:, K - 1] - DELTA
    cand = (part[:, :, None] + POOLW * np.arange(EXPAND)[None, None, :]
            ).reshape(M, T_CAND * EXPAND)
    idx = np.empty((M, K), np.int64)
    BS = 2048
    for s in range(0, M, BS):
        e = min(s + BS, M)
        cb = cand[s:e]
        g = xb[cb]
        dots = np.matmul(g, xb[s:e, :, None])[..., 0].astype(np.float32)
        d2c = x2f[s:e, None] + x2f[cb] - 2.0 * dots
        order = np.lexsort((cb, d2c), axis=1)[:, :K]
        idx[s:e] = np.take_along_axis(cb, order, axis=1)
    return idx, suspect


def _host_topk_row(xb, x2f, r):
    d2 = x2f + x2f[r] - 2.0 * (xb @ xb[r]).astype(np.float32)
    order = np.lexsort((np.arange(M), d2))[:K]
    return order


def kernel(x, k):
    x = np.asarray(x, dtype=np.float32)
    k = int(k)
    assert x.shape == (N, M, D) and k == K

    from concourse.bass_utils import run_bass_kernel_spmd

    if "nc" not in _COMPILED:
        _COMPILED["nc"] = _build_nc()
    nc = _COMPILED["nc"]

    in_maps = _prep_inputs(x)
    res = run_bass_kernel_spmd(nc, in_maps, list(range(NCORES))).results

    p1 = np.empty((N, M, POOLW), np.float16)
    for c in range(NCORES):
        b, h = c // 2, c % 2
        sl = slice(h * QROWS, (h + 1) * QROWS)
        p1[b, sl] = res[c]["p1"].reshape(QROWS, POOLW)

    idx = np.empty((N, M, K), np.int64)
    for b in range(N):
        idx_b, suspect = _rerank_batch(x[b], p1[b])
        idx[b] = idx_b
        rows = np.nonzero(suspect)[0]
        if rows.size:
            x2f = (x[b] * x[b]).sum(-1)
            for r in rows:
                idx[b, r] = _host_topk_row(x[b], x2f, r)

    offset = (np.arange(N, dtype=np.int64) * M)[:, None, None]
    src = (idx + offset).reshape(-1).astype(np.int32)
    dst = np.repeat(np.arange(N * M, dtype=np.int32), K)
    return src, dst


if __name__ == "__main__":
    rng = np.random.default_rng(0)
    xt = rng.standard_normal((N, M, D), dtype=np.float32)
    s, d = kernel(xt, 16)
    print(s[:32], d[:32])


# revision 5
# speedup vs baseline: 1.0118x; 1.0118x over previous
"""KNNGraph v5 — v4 + interleaved kv tiles (fast start) + pb-first issue order.

Per core, per group of 128 query rows (32 groups):
  PE:  16 fp32r matmuls -> 8 PSUM blocks (128, 1024) of w = q.k - |k|^2/2
  Exit+pool per block-pair d (keys [1024d,1024d+1024) with +4096):
    'Y': ACT converts pb -> fp16 SBUF; DVE fused max(pa PSUM fp32, wb fp16)
    'X' (every X_DUTY-th group, pair 0): ACT converts both; DVE fp16 max
  DMA: ship p1 (128, 4096) fp16 per group.
Host: top-T pooled per row, expand x2 (keys j, j+4096), exact fp32 re-rank;
near-tie suspect rows -> exact full-row fallback.

kv is host-permuted into 4 tiles of [128, 2048]: tile d = keys
[1024d:1024d+1024) ++ [4096+1024d : 4096+1024d+1024), so each block-pair
depends on one 1MB DMA. q is split into 4 tiles of 8 groups each.
"""

import numpy as np

N, M, D = 4, 8192, 64
K = 16
NCORES = 8
QROWS = M // 2
NGROUPS = QROWS // 128  # 32
BLK = 1024
NBLK = 4
KDIM = 128
POOLW = M // 2          # 4096
EXPAND = 2
T_CAND = 24
DELTA = 0.25
X_DUTY = 0

_COMPILED = {}


def _is_x_pair(g, d, x_duty):
    # all-'Y' measured fastest: every X swap stalls the pipeline more than the
    # ACT/DVE busy-rebalance saves
    return bool(x_duty) and g % x_duty == x_duty - 1 and d == 2


def _build_nc(x_duty=None):
    import concourse.bacc as bacc
    import concourse.mybir as mybir
    import concourse.tile as tile

    x_duty = x_duty if x_duty is not None else X_DUTY

    nc = bacc.Bacc(None)
    f32 = mybir.dt.float32
    f32r = mybir.dt.float32r
    f16 = mybir.dt.float16

    q_d = nc.declare_dram_parameter("q", [KDIM, QROWS], f32r, isOutput=False)
    kv_d = nc.declare_dram_parameter("kv", [KDIM, M], f32r, isOutput=False)
    p1_d = nc.declare_dram_parameter("p1", [NGROUPS, 128, POOLW], f16, isOutput=True)

    with tile.TileContext(nc) as tc:
        with (
            tc.tile_pool(name="singles", bufs=1) as singles,
            tc.tile_pool(name="psum", bufs=2, space="PSUM") as psum,
            tc.tile_pool(name="w16", bufs=8) as w16pool,
            tc.tile_pool(name="pool1", bufs=6) as p1pool,
        ):
            NQ = 8
            q_sb = [singles.tile([KDIM, QROWS // NQ], f32r, name=f"q_sb{i}")
                    for i in range(NQ)]
            # kv tile d split into pb part (cols BLK..2BLK, needed first) and pa
            kv_sb = [singles.tile([KDIM, 2 * BLK], f32r, name=f"kv_sb{i}")
                     for i in range(NBLK)]
            # critical-path order on the hw queue: q tile 0, kv0 pb-half, kv0 pa-half
            nc.sync.dma_start(out=q_sb[0][:], in_=q_d[:, 0:QROWS // NQ])
            nc.sync.dma_start(out=kv_sb[0][:, BLK:2 * BLK], in_=kv_d[:, BLK:2 * BLK])
            nc.sync.dma_start(out=kv_sb[0][:, 0:BLK], in_=kv_d[:, 0:BLK])
            for d in range(1, NBLK):
                nc.sync.dma_start(out=kv_sb[d][:],
                                  in_=kv_d[:, 2 * BLK * d:2 * BLK * (d + 1)])
            for i in range(1, NQ):
                nc.sync.dma_start(
                    out=q_sb[i][:], in_=q_d[:, i * (QROWS // NQ):(i + 1) * (QROWS // NQ)])

            GPQ = NGROUPS // NQ  # groups per q tile
            for g in range(NGROUPS):
                qi, qr = g // GPQ, g % GPQ
                lhsT = q_sb[qi][:, qr * 128:(qr + 1) * 128]
                p1 = p1pool.tile([128, POOLW], f16, tag="p1")
                for d in range(NBLK):
                    pa = psum.tile([128, BLK], f32, tag="pa")
                    pb = psum.tile([128, BLK], f32, tag="pb")
                    for half in range(2):
                        nc.tensor.matmul(
                            pb[:, half * 512:(half + 1) * 512], lhsT,
                            kv_sb[d][:, BLK + half * 512:BLK + (half + 1) * 512],
                            start=True, stop=True)
                    wb = w16pool.tile([128, BLK], f16, tag="wb")
                    nc.scalar.copy(out=wb[:], in_=pb[:])
                    for half in range(2):
                        nc.tensor.matmul(
                            pa[:, half * 512:(half + 1) * 512], lhsT,
                            kv_sb[d][:, half * 512:(half + 1) * 512],
                            start=True, stop=True)
                    out_sl = p1[:, d * BLK:(d + 1) * BLK]
                    if _is_x_pair(g, d, x_duty):
                        wa = w16pool.tile([128, BLK], f16, tag="wa")
                        nc.scalar.copy(out=wa[:], in_=pa[:])
                        nc.vector.tensor_max(out_sl, wa[:], wb[:])
                    else:
                        nc.vector.tensor_max(out_sl, pa[:], wb[:])
                nparts = 4 if g == NGROUPS - 1 else 2
                for pp in range(nparts):
                    lo, hi = pp * POOLW // nparts, (pp + 1) * POOLW // nparts
                    nc.sync.dma_start(out=p1_d[g, :, lo:hi], in_=p1[:, lo:hi])
    if not nc.is_finalized():
        nc.finalize()
    return nc


def _prep_inputs(x):
    x64 = x.astype(np.float64)
    x2 = (x64 * x64).sum(-1)
    neg_half_x2 = (-0.5 * x2).astype(np.float32)
    in_maps = []
    for c in range(NCORES):
        b, h = c // 2, c % 2
        q = np.zeros((KDIM, QROWS), np.float32)
        q[:D] = x[b, h * QROWS:(h + 1) * QROWS, :].T
        q[D] = 1.0
        kv = np.zeros((KDIM, M), np.float32)
        kv[:D] = x[b].T
        kv[D] = neg_half_x2[b]
        # permute into 4 tiles: tile d = cols [1024d:1024d+1024) ++ [4096+1024d:...)
        kvp = np.empty_like(kv)
        for d in range(NBLK):
            kvp[:, 2048 * d:2048 * d + 1024] = kv[:, 1024 * d:1024 * (d + 1)]
            kvp[:, 2048 * d + 1024:2048 * (d + 1)] = kv[:, POOLW + 1024 * d:POOLW + 1024 * (d + 1)]
        in_maps.append({"q": q, "kv": kvp})
    return in_maps


def _rerank_batch(xb, p1b):
    x2f = (xb * xb).sum(-1)
    p = p1b.astype(np.float32)
    np.nan_to_num(p, copy=False, nan=-np.inf)
    part = np.argpartition(-p, T_CAND - 1, axis=1)[:, :T_CAND]
    pv = np.take_along_axis(p, part, axis=1)
    pvs = -np.sort(-pv, axis=1)
    suspect = pvs[:, T_CAND - 1] >= pvs[:, K - 1] - DELTA
    cand = (part[:, :, None] + POOLW * np.arange(EXPAND)[None, None, :]
            ).reshape(M, T_CAND * EXPAND)
    idx = np.empty((M, K), np.int64)
    BS = 2048
    for s in range(0, M, BS):
        e = min(s + BS, M)
        cb = cand[s:e]
        g = xb[cb]
        dots = np.matmul(g, xb[s:e, :, None])[..., 0].astype(np.float32)
        d2c = x2f[s:e, None] + x2f[cb] - 2.0 * dots
        order = np.lexsort((cb, d2c), axis=1)[:, :K]
        idx[s:e] = np.take_along_axis(cb, order, axis=1)
    return idx, suspect


def _host_topk_row(xb, x2f, r):
    d2 = x2f + x2f[r] - 2.0 * (xb @ xb[r]).astype(np.float32)
    order = np.lexsort((np.arange(M), d2))[:K]
    return order


def kernel(x, k):
    x = np.asarray(x, dtype=np.float32)
    k = int(k)
    assert x.shape == (N, M, D) and k == K

    from concourse.bass_utils import run_bass_kernel_spmd

    if "nc" not in _COMPILED:
        _COMPILED["nc"] = _build_nc()
    nc = _COMPILED["nc"]

    in_maps = _prep_inputs(x)
    res = run_bass_kernel_spmd(nc, in_maps, list(range(NCORES))).results

    p1 = np.empty((N, M, POOLW), np.float16)
    for c in range(NCORES):
        b, h = c // 2, c % 2
        sl = slice(h * QROWS, (h + 1) * QROWS)
        p1[b, sl] = res[c]["p1"].reshape(QROWS, POOLW)

    idx = np.empty((N, M, K), np.int64)
    for b in range(N):
        idx_b, suspect = _rerank_batch(x[b], p1[b])
        idx[b] = idx_b
        rows = np.nonzero(suspect)[0]
        if rows.size:
            x2f = (x[b] * x[b]).sum(-1)
            for r in rows:
                idx[b, r] = _host_topk_row(x[b], x2f, r)

    offset = (np.arange(N, dtype=np.int64) * M)[:, None, None]
    src = (idx + offset).reshape(-1).astype(np.int32)
    dst = np.repeat(np.arange(N * M, dtype=np.int32), K)
    return src, dst


if __name__ == "__main__":
    rng = np.random.default_rng(0)
    xt = rng.standard_normal((N, M, D), dtype=np.float32)
    s, d = kernel(xt, 16)
    print(s[:32], d[:32])


# revision 6
# speedup vs baseline: 1.0341x; 1.0220x over previous
"""KNNGraph v5 — v4 + interleaved kv tiles (fast start) + pb-first issue order.

Per core, per group of 128 query rows (32 groups):
  PE:  16 fp32r matmuls -> 8 PSUM blocks (128, 1024) of w = q.k - |k|^2/2
  Exit+pool per block-pair d (keys [1024d,1024d+1024) with +4096):
    'Y': ACT converts pb -> fp16 SBUF; DVE fused max(pa PSUM fp32, wb fp16)
    'X' (every X_DUTY-th group, pair 0): ACT converts both; DVE fp16 max
  DMA: ship p1 (128, 4096) fp16 per group.
Host: top-T pooled per row, expand x2 (keys j, j+4096), exact fp32 re-rank;
near-tie suspect rows -> exact full-row fallback.

kv is host-permuted into 4 tiles of [128, 2048]: tile d = keys
[1024d:1024d+1024) ++ [4096+1024d : 4096+1024d+1024), so each block-pair
depends on one 1MB DMA. q is split into 4 tiles of 8 groups each.
"""

import numpy as np

N, M, D = 4, 8192, 64
K = 16
NCORES = 8
QROWS = M // 2
NGROUPS = QROWS // 128  # 32
BLK = 1024
NBLK = 4
KDIM = 65            # contraction rows actually used (64 dims + ones row)
POOLW = M // 2          # 4096
EXPAND = 2
T_CAND = 24
DELTA = 0.25
X_DUTY = 0

_COMPILED = {}


def _is_x_pair(g, d, x_duty):
    # all-'Y' measured fastest: every X swap stalls the pipeline more than the
    # ACT/DVE busy-rebalance saves
    return bool(x_duty) and g % x_duty == x_duty - 1 and d == 2


def _build_nc(x_duty=None):
    import concourse.bacc as bacc
    import concourse.mybir as mybir
    import concourse.tile as tile

    x_duty = x_duty if x_duty is not None else X_DUTY

    nc = bacc.Bacc(None)
    f32 = mybir.dt.float32
    f32r = mybir.dt.float32r
    f16 = mybir.dt.float16

    q_d = nc.declare_dram_parameter("q", [KDIM, QROWS], f32r, isOutput=False)
    kv_d = nc.declare_dram_parameter("kv", [KDIM, M], f32r, isOutput=False)
    p1_d = nc.declare_dram_parameter("p1", [NGROUPS, 128, POOLW], f16, isOutput=True)

    with tile.TileContext(nc) as tc:
        with (
            tc.tile_pool(name="singles", bufs=1) as singles,
            tc.tile_pool(name="psum", bufs=2, space="PSUM") as psum,
            tc.tile_pool(name="w16", bufs=8) as w16pool,
            tc.tile_pool(name="pool1", bufs=6) as p1pool,
        ):
            NQ = 8
            q_sb = [singles.tile([KDIM, QROWS // NQ], f32r, name=f"q_sb{i}")
                    for i in range(NQ)]
            # kv tile d split into pb part (cols BLK..2BLK, needed first) and pa
            kv_sb = [singles.tile([KDIM, 2 * BLK], f32r, name=f"kv_sb{i}")
                     for i in range(NBLK)]
            # critical-path order on the hw queue: q tile 0, kv0 pb-half, kv0 pa-half
            nc.sync.dma_start(out=q_sb[0][:], in_=q_d[:, 0:QROWS // NQ])
            nc.sync.dma_start(out=kv_sb[0][:, BLK:2 * BLK], in_=kv_d[:, BLK:2 * BLK])
            nc.sync.dma_start(out=kv_sb[0][:, 0:BLK], in_=kv_d[:, 0:BLK])
            for d in range(1, NBLK):
                nc.sync.dma_start(out=kv_sb[d][:],
                                  in_=kv_d[:, 2 * BLK * d:2 * BLK * (d + 1)])
            for i in range(1, NQ):
                nc.sync.dma_start(
                    out=q_sb[i][:], in_=q_d[:, i * (QROWS // NQ):(i + 1) * (QROWS // NQ)])

            GPQ = NGROUPS // NQ  # groups per q tile
            for g in range(NGROUPS):
                qi, qr = g // GPQ, g % GPQ
                lhsT = q_sb[qi][:, qr * 128:(qr + 1) * 128]
                p1 = p1pool.tile([128, POOLW], f16, tag="p1")
                for d in range(NBLK):
                    pa = psum.tile([128, BLK], f32, tag="pa")
                    pb = psum.tile([128, BLK], f32, tag="pb")
                    for half in range(2):
                        nc.tensor.matmul(
                            pb[:, half * 512:(half + 1) * 512], lhsT,
                            kv_sb[d][:, BLK + half * 512:BLK + (half + 1) * 512],
                            start=True, stop=True)
                    wb = w16pool.tile([128, BLK], f16, tag="wb")
                    nc.scalar.copy(out=wb[:], in_=pb[:])
                    for half in range(2):
                        nc.tensor.matmul(
                            pa[:, half * 512:(half + 1) * 512], lhsT,
                            kv_sb[d][:, half * 512:(half + 1) * 512],
                            start=True, stop=True)
                    out_sl = p1[:, d * BLK:(d + 1) * BLK]
                    if _is_x_pair(g, d, x_duty):
                        wa = w16pool.tile([128, BLK], f16, tag="wa")
                        nc.scalar.copy(out=wa[:], in_=pa[:])
                        nc.vector.tensor_max(out_sl, wa[:], wb[:])
                    else:
                        nc.vector.tensor_max(out_sl, pa[:], wb[:])
                nparts = 4 if g == NGROUPS - 1 else 2
                for pp in range(nparts):
                    lo, hi = pp * POOLW // nparts, (pp + 1) * POOLW // nparts
                    nc.sync.dma_start(out=p1_d[g, :, lo:hi], in_=p1[:, lo:hi])
    if not nc.is_finalized():
        nc.finalize()
    return nc


def _prep_inputs(x):
    x64 = x.astype(np.float64)
    x2 = (x64 * x64).sum(-1)
    neg_half_x2 = (-0.5 * x2).astype(np.float32)
    in_maps = []
    for c in range(NCORES):
        b, h = c // 2, c % 2
        q = np.zeros((KDIM, QROWS), np.float32)
        q[:D] = x[b, h * QROWS:(h + 1) * QROWS, :].T
        q[D] = 1.0
        kv = np.zeros((KDIM, M), np.float32)
        kv[:D] = x[b].T
        kv[D] = neg_half_x2[b]
        # permute into 4 tiles: tile d = cols [1024d:1024d+1024) ++ [4096+1024d:...)
        kvp = np.empty_like(kv)
        for d in range(NBLK):
            kvp[:, 2048 * d:2048 * d + 1024] = kv[:, 1024 * d:1024 * (d + 1)]
            kvp[:, 2048 * d + 1024:2048 * (d + 1)] = kv[:, POOLW + 1024 * d:POOLW + 1024 * (d + 1)]
        in_maps.append({"q": q, "kv": kvp})
    return in_maps


def _rerank_batch(xb, p1b):
    x2f = (xb * xb).sum(-1)
    p = p1b.astype(np.float32)
    np.nan_to_num(p, copy=False, nan=-np.inf)
    part = np.argpartition(-p, T_CAND - 1, axis=1)[:, :T_CAND]
    pv = np.take_along_axis(p, part, axis=1)
    pvs = -np.sort(-pv, axis=1)
    suspect = pvs[:, T_CAND - 1] >= pvs[:, K - 1] - DELTA
    cand = (part[:, :, None] + POOLW * np.arange(EXPAND)[None, None, :]
            ).reshape(M, T_CAND * EXPAND)
    idx = np.empty((M, K), np.int64)
    BS = 2048
    for s in range(0, M, BS):
        e = min(s + BS, M)
        cb = cand[s:e]
        g = xb[cb]
        dots = np.matmul(g, xb[s:e, :, None])[..., 0].astype(np.float32)
        d2c = x2f[s:e, None] + x2f[cb] - 2.0 * dots
        order = np.lexsort((cb, d2c), axis=1)[:, :K]
        idx[s:e] = np.take_along_axis(cb, order, axis=1)
    return idx, suspect


def _host_topk_row(xb, x2f, r):
    d2 = x2f + x2f[r] - 2.0 * (xb @ xb[r]).astype(np.float32)
    order = np.lexsort((np.arange(M), d2))[:K]
    return order


def kernel(x, k):
    x = np.asarray(x, dtype=np.float32)
    k = int(k)
    assert x.shape == (N, M, D) and k == K

    from concourse.bass_utils import run_bass_kernel_spmd

    if "nc" not in _COMPILED:
        _COMPILED["nc"] = _build_nc()
    nc = _COMPILED["nc"]

    in_maps = _prep_inputs(x)
    res = run_bass_kernel_spmd(nc, in_maps, list(range(NCORES))).results

    p1 = np.empty((N, M, POOLW), np.float16)
    for c in range(NCORES):
        b, h = c // 2, c % 2
        sl = slice(h * QROWS, (h + 1) * QROWS)
        p1[b, sl] = res[c]["p1"].reshape(QROWS, POOLW)

    idx = np.empty((N, M, K), np.int64)
    for b in range(N):
        idx_b, suspect = _rerank_batch(x[b], p1[b])
        idx[b] = idx_b
        rows = np.nonzero(suspect)[0]
        if rows.size:
            x2f = (x[b] * x[b]).sum(-1)
            for r in rows:
                idx[b, r] = _host_topk_row(x[b], x2f, r)

    offset = (np.arange(N, dtype=np.int64) * M)[:, None, None]
    src = (idx + offset).reshape(-1).astype(np.int32)
    dst = np.repeat(np.arange(N * M, dtype=np.int32), K)
    return src, dst


if __name__ == "__main__":
    rng = np.random.default_rng(0)
    xt = rng.standard_normal((N, M, D), dtype=np.float32)
    s, d = kernel(xt, 16)
    print(s[:32], d[:32])


# revision 8
# speedup vs baseline: 1.0414x; 1.0071x over previous
"""KNNGraph (k=16) Bass kernel for 8 NeuronCores — pooled-candidate design.

Input: x (4, 8192, 64) fp32. Output: (src, dst) int32 edge arrays of the
16-NN graph per batch (self included), matching jax.lax.top_k(-d2) order.

Sharding: core c handles batch c//2, query rows (c%2)*4096 ... +4096,
against all 8192 keys of that batch (query-row sharding, keys replicated).

Per core, per group of 128 query rows (32 groups):
  PE:  16 fp32r matmuls (K=65: 64 dims + ones row folding -|key|^2/2)
       -> 8 PSUM blocks (128, 1024) of w = q.k - |k|^2/2 (rank-equiv to -d2/2)
  Exit+pool per block-pair d (keys [1024d, 1024d+1024) paired with +4096):
    'Y': ACT converts pb -> fp16 SBUF; DVE fused max(pa PSUM fp32, wb fp16)
    'X' (final drain region only): ACT converts both; DVE fp16 max (2x mode)
  DMA: ship p1 (128, 4096) fp16 per group.
Host: top-T_CAND pooled positions per row (argpartition), expand x2 (pooled j
-> keys {j, j+4096}), exact reference-style fp32 re-rank; rows with a
near-tie at the candidate cut get an exact full-row fallback.

kv is host-permuted into 4 tiles of [65, 2048]: tile d = keys
[1024d:1024d+1024) ++ [4096+1024d:...+1024), so each block-pair depends on a
single 0.5MB DMA; q is split into 8 tiles so group 0 starts after ~1MB of DMA.

Engine budget per core (cost model): DVE 153us (128 fused exits), ACT 134us,
PE 113us, DMA 116us; total ~165us vs 923us for the direct top-k baseline.
"""

import numpy as np

N, M, D = 4, 8192, 64
K = 16
NCORES = 8
QROWS = M // 2
NGROUPS = QROWS // 128  # 32
BLK = 1024
NBLK = 4
KDIM = 65            # contraction rows actually used (64 dims + ones row)
POOLW = M // 2          # 4096
EXPAND = 2
T_CAND = 24
DELTA = 0.25
X_DUTY = 0

_COMPILED = {}


_X_PAIRS = {(28, 3), (29, 1), (29, 3), (30, 1), (30, 3), (31, 1), (31, 3)}


def _is_x_pair(g, d, x_duty):
    # 'X' only in the final drain region: DVE (1192ns/pair) runs ~150ns/pair
    # behind ACT (1038ns/pair) all kernel long; swapping the last few pairs to
    # the ACT-heavy form lets both engines finish together. Steady-state X
    # swaps measured slower (pipeline stalls outweigh the rebalance).
    return (g, d) in _X_PAIRS


def _build_nc(x_duty=None):
    import concourse.bacc as bacc
    import concourse.mybir as mybir
    import concourse.tile as tile

    x_duty = x_duty if x_duty is not None else X_DUTY

    nc = bacc.Bacc(None)
    f32 = mybir.dt.float32
    f32r = mybir.dt.float32r
    f16 = mybir.dt.float16

    q_d = nc.declare_dram_parameter("q", [KDIM, QROWS], f32r, isOutput=False)
    kv_d = nc.declare_dram_parameter("kv", [KDIM, M], f32r, isOutput=False)
    p1_d = nc.declare_dram_parameter("p1", [NGROUPS, 128, POOLW], f16, isOutput=True)

    with tile.TileContext(nc) as tc:
        with (
            tc.tile_pool(name="singles", bufs=1) as singles,
            tc.tile_pool(name="psum", bufs=2, space="PSUM") as psum,
            tc.tile_pool(name="w16", bufs=8) as w16pool,
            tc.tile_pool(name="pool1", bufs=6) as p1pool,
        ):
            NQ = 8
            q_sb = [singles.tile([KDIM, QROWS // NQ], f32r, name=f"q_sb{i}")
                    for i in range(NQ)]
            # kv tile d split into pb part (cols BLK..2BLK, needed first) and pa
            kv_sb = [singles.tile([KDIM, 2 * BLK], f32r, name=f"kv_sb{i}")
                     for i in range(NBLK)]
            # critical-path order on the hw queue: q tile 0, kv0 pb-half, kv0 pa-half
            nc.sync.dma_start(out=q_sb[0][:], in_=q_d[:, 0:QROWS // NQ])
            nc.sync.dma_start(out=kv_sb[0][:, BLK:2 * BLK], in_=kv_d[:, BLK:2 * BLK])
            nc.sync.dma_start(out=kv_sb[0][:, 0:BLK], in_=kv_d[:, 0:BLK])
            for d in range(1, NBLK):
                nc.sync.dma_start(out=kv_sb[d][:],
                                  in_=kv_d[:, 2 * BLK * d:2 * BLK * (d + 1)])
            for i in range(1, NQ):
                nc.sync.dma_start(
                    out=q_sb[i][:], in_=q_d[:, i * (QROWS // NQ):(i + 1) * (QROWS // NQ)])

            GPQ = NGROUPS // NQ  # groups per q tile
            for g in range(NGROUPS):
                qi, qr = g // GPQ, g % GPQ
                lhsT = q_sb[qi][:, qr * 128:(qr + 1) * 128]
                p1 = p1pool.tile([128, POOLW], f16, tag="p1")
                for d in range(NBLK):
                    pa = psum.tile([128, BLK], f32, tag="pa")
                    pb = psum.tile([128, BLK], f32, tag="pb")
                    for half in range(2):
                        nc.tensor.matmul(
                            pb[:, half * 512:(half + 1) * 512], lhsT,
                            kv_sb[d][:, BLK + half * 512:BLK + (half + 1) * 512],
                            start=True, stop=True)
                    wb = w16pool.tile([128, BLK], f16, tag="wb")
                    nc.scalar.copy(out=wb[:], in_=pb[:])
                    for half in range(2):
                        nc.tensor.matmul(
                            pa[:, half * 512:(half + 1) * 512], lhsT,
                            kv_sb[d][:, half * 512:(half + 1) * 512],
                            start=True, stop=True)
                    out_sl = p1[:, d * BLK:(d + 1) * BLK]
                    if _is_x_pair(g, d, x_duty):
                        wa = w16pool.tile([128, BLK], f16, tag="wa")
                        nc.scalar.copy(out=wa[:], in_=pa[:])
                        nc.vector.tensor_max(out_sl, wa[:], wb[:])
                    else:
                        nc.vector.tensor_max(out_sl, pa[:], wb[:])
                nparts = 4 if g == NGROUPS - 1 else 2
                for pp in range(nparts):
                    lo, hi = pp * POOLW // nparts, (pp + 1) * POOLW // nparts
                    nc.sync.dma_start(out=p1_d[g, :, lo:hi], in_=p1[:, lo:hi])
    if not nc.is_finalized():
        nc.finalize()
    return nc


def _prep_inputs(x):
    x64 = x.astype(np.float64)
    x2 = (x64 * x64).sum(-1)
    neg_half_x2 = (-0.5 * x2).astype(np.float32)
    in_maps = []
    for c in range(NCORES):
        b, h = c // 2, c % 2
        q = np.zeros((KDIM, QROWS), np.float32)
        q[:D] = x[b, h * QROWS:(h + 1) * QROWS, :].T
        q[D] = 1.0
        kv = np.zeros((KDIM, M), np.float32)
        kv[:D] = x[b].T
        kv[D] = neg_half_x2[b]
        # permute into 4 tiles: tile d = cols [1024d:1024d+1024) ++ [4096+1024d:...)
        kvp = np.empty_like(kv)
        for d in range(NBLK):
            kvp[:, 2048 * d:2048 * d + 1024] = kv[:, 1024 * d:1024 * (d + 1)]
            kvp[:, 2048 * d + 1024:2048 * (d + 1)] = kv[:, POOLW + 1024 * d:POOLW + 1024 * (d + 1)]
        in_maps.append({"q": q, "kv": kvp})
    return in_maps


def _rerank_batch(xb, p1b):
    x2f = (xb * xb).sum(-1)
    p = p1b.astype(np.float32)
    np.nan_to_num(p, copy=False, nan=-np.inf)
    part = np.argpartition(-p, T_CAND - 1, axis=1)[:, :T_CAND]
    pv = np.take_along_axis(p, part, axis=1)
    pvs = -np.sort(-pv, axis=1)
    suspect = pvs[:, T_CAND - 1] >= pvs[:, K - 1] - DELTA
    cand = (part[:, :, None] + POOLW * np.arange(EXPAND)[None, None, :]
            ).reshape(M, T_CAND * EXPAND)
    idx = np.empty((M, K), np.int64)
    BS = 2048
    for s in range(0, M, BS):
        e = min(s + BS, M)
        cb = cand[s:e]
        g = xb[cb]
        dots = np.matmul(g, xb[s:e, :, None])[..., 0].astype(np.float32)
        d2c = x2f[s:e, None] + x2f[cb] - 2.0 * dots
        order = np.lexsort((cb, d2c), axis=1)[:, :K]
        idx[s:e] = np.take_along_axis(cb, order, axis=1)
    return idx, suspect


def _host_topk_row(xb, x2f, r):
    d2 = x2f + x2f[r] - 2.0 * (xb @ xb[r]).astype(np.float32)
    order = np.lexsort((np.arange(M), d2))[:K]
    return order


def kernel(x, k):
    x = np.asarray(x, dtype=np.float32)
    k = int(k)
    assert x.shape == (N, M, D) and k == K

    from concourse.bass_utils import run_bass_kernel_spmd

    if "nc" not in _COMPILED:
        _COMPILED["nc"] = _build_nc()
    nc = _COMPILED["nc"]

    in_maps = _prep_inputs(x)
    res = run_bass_kernel_spmd(nc, in_maps, list(range(NCORES))).results

    p1 = np.empty((N, M, POOLW), np.float16)
    for c in range(NCORES):
        b, h = c // 2, c % 2
        sl = slice(h * QROWS, (h + 1) * QROWS)
        p1[b, sl] = res[c]["p1"].reshape(QROWS, POOLW)

    idx = np.empty((N, M, K), np.int64)
    for b in range(N):
        idx_b, suspect = _rerank_batch(x[b], p1[b])
        idx[b] = idx_b
        rows = np.nonzero(suspect)[0]
        if rows.size:
            x2f = (x[b] * x[b]).sum(-1)
            for r in rows:
                idx[b, r] = _host_topk_row(x[b], x2f, r)

    offset = (np.arange(N, dtype=np.int64) * M)[:, None, None]
    src = (idx + offset).reshape(-1).astype(np.int32)
    dst = np.repeat(np.arange(N * M, dtype=np.int32), K)
    return src, dst


if __name__ == "__main__":
    rng = np.random.default_rng(0)
    xt = rng.standard_normal((N, M, D), dtype=np.float32)
    s, d = kernel(xt, 16)
    print(s[:32], d[:32])


# revision 11
# speedup vs baseline: 1.0637x; 1.0215x over previous
"""KNNGraph (k=16) Bass kernel for 8 NeuronCores — pooled-candidate design.

Input: x (4, 8192, 64) fp32. Output: (src, dst) int32 edge arrays of the
16-NN graph per batch (self included), matching jax.lax.top_k(-d2) order.

Sharding: core c handles batch c//2, query rows (c%2)*4096 ... +4096,
against all 8192 keys of that batch (query-row sharding, keys replicated).

Per core, per group of 128 query rows (32 groups):
  PE:  16 fp32r matmuls (K=65: 64 dims + ones row folding -|key|^2/2)
       -> 8 PSUM blocks (128, 1024) of w = q.k - |k|^2/2 (rank-equiv to -d2/2)
  Exit+pool per block-pair d (keys [1024d, 1024d+1024) paired with +4096):
    'Y': ACT converts pb -> fp16 SBUF; DVE fused max(pa PSUM fp32, wb fp16)
    'X' (final drain region only): ACT converts both; DVE fp16 max (2x mode)
  DMA: ship p1 (128, 4096) fp16 per group.
Host: top-T_CAND pooled positions per row (argpartition), expand x2 (pooled j
-> keys {j, j+4096}), exact reference-style fp32 re-rank; rows with a
near-tie at the candidate cut get an exact full-row fallback.

kv is host-permuted into 4 tiles of [65, 2048]: tile d = keys
[1024d:1024d+1024) ++ [4096+1024d:...+1024), so each block-pair depends on a
single 0.5MB DMA; q is split into 8 tiles so group 0 starts after ~1MB of DMA.

Engine budget per core (cost model): DVE 153us (128 fused exits, saturated),
ACT 134us, PE 113us, DMA 116us; total ~165us vs 923us for the direct top-k
baseline. The schedule is exit-chained: pa PSUM tiles are freed by the DVE
exits, so the steady-state period equals the DVE exit time; emission order
and priorities don't matter (the Tile scheduler rebuilds order from the
dependency graph).
"""

import numpy as np

N, M, D = 4, 8192, 64
K = 16
NCORES = 8
QROWS = M // 2
NGROUPS = QROWS // 128  # 32
BLK = 1024
NBLK = 4
KDIM = 65            # contraction rows actually used (64 dims + ones row)
POOLW = M // 2          # 4096
EXPAND = 2
T_CAND = 24
DELTA = 0.25
X_DUTY = 0

_COMPILED = {}


_X_PAIRS = {(28, 3), (29, 1), (29, 3), (30, 1), (30, 3), (31, 1), (31, 3)}


def _is_x_pair(g, d, x_duty):
    # 'X' only in the final drain region: DVE (1192ns/pair) runs ~150ns/pair
    # behind ACT (1038ns/pair) all kernel long; swapping the last few pairs to
    # the ACT-heavy form lets both engines finish together. Steady-state X
    # swaps measured slower (pipeline stalls outweigh the rebalance).
    return (g, d) in _X_PAIRS


def _build_nc(x_duty=None):
    import concourse.bacc as bacc
    import concourse.mybir as mybir
    import concourse.tile as tile

    x_duty = x_duty if x_duty is not None else X_DUTY

    nc = bacc.Bacc(None)
    f32 = mybir.dt.float32
    f32r = mybir.dt.float32r
    f16 = mybir.dt.float16

    q_d = nc.declare_dram_parameter("q", [KDIM, QROWS], f32r, isOutput=False)
    kv_d = nc.declare_dram_parameter("kv", [KDIM, M], f32r, isOutput=False)
    p1_d = nc.declare_dram_parameter("p1", [NGROUPS, 128, POOLW], f16, isOutput=True)

    with tile.TileContext(nc) as tc:
        with (
            tc.tile_pool(name="singles", bufs=1) as singles,
            tc.tile_pool(name="psum", bufs=2, space="PSUM") as psum,
            tc.tile_pool(name="w16", bufs=8) as w16pool,
            tc.tile_pool(name="pool1", bufs=6) as p1pool,
        ):
            NQ = 8
            q_sb = [singles.tile([KDIM, QROWS // NQ], f32r, name=f"q_sb{i}")
                    for i in range(NQ)]
            # kv tile d split into pb part (cols BLK..2BLK, needed first) and pa
            kv_sb = [singles.tile([KDIM, 2 * BLK], f32r, name=f"kv_sb{i}")
                     for i in range(NBLK)]
            # critical-path order on the hw queue: q tile 0, kv0 pb-half, kv0 pa-half
            nc.sync.dma_start(out=q_sb[0][:], in_=q_d[:, 0:QROWS // NQ])
            nc.sync.dma_start(out=kv_sb[0][:, BLK:2 * BLK], in_=kv_d[:, BLK:2 * BLK])
            nc.sync.dma_start(out=kv_sb[0][:, 0:BLK], in_=kv_d[:, 0:BLK])
            for d in range(1, NBLK):
                nc.sync.dma_start(out=kv_sb[d][:],
                                  in_=kv_d[:, 2 * BLK * d:2 * BLK * (d + 1)])
            for i in range(1, NQ):
                nc.sync.dma_start(
                    out=q_sb[i][:], in_=q_d[:, i * (QROWS // NQ):(i + 1) * (QROWS // NQ)])

            GPQ = NGROUPS // NQ  # groups per q tile
            for g in range(NGROUPS):
                qi, qr = g // GPQ, g % GPQ
                lhsT = q_sb[qi][:, qr * 128:(qr + 1) * 128]
                p1 = p1pool.tile([128, POOLW], f16, tag="p1")
                for d in range(NBLK):
                    pa = psum.tile([128, BLK], f32, tag="pa")
                    pb = psum.tile([128, BLK], f32, tag="pb")
                    out_sl = p1[:, d * BLK:(d + 1) * BLK]
                    is_x = _is_x_pair(g, d, x_duty)

                    def mm(dst, base):
                        for half in range(2):
                            nc.tensor.matmul(
                                dst[:, half * 512:(half + 1) * 512], lhsT,
                                kv_sb[d][:, base + half * 512:base + (half + 1) * 512],
                                start=True, stop=True)

                    if is_x:
                        # pa first so its ACT convert (which frees the pa PSUM
                        # tile) is ready early in the ACT stream
                        mm(pa, 0)
                        wa = w16pool.tile([128, BLK], f16, tag="wa")
                        nc.scalar.copy(out=wa[:], in_=pa[:])
                        mm(pb, BLK)
                        wb = w16pool.tile([128, BLK], f16, tag="wb")
                        nc.scalar.copy(out=wb[:], in_=pb[:])
                        nc.vector.tensor_max(out_sl, wa[:], wb[:])
                    else:
                        mm(pb, BLK)
                        wb = w16pool.tile([128, BLK], f16, tag="wb")
                        nc.scalar.copy(out=wb[:], in_=pb[:])
                        mm(pa, 0)
                        nc.vector.tensor_max(out_sl, pa[:], wb[:])
                nparts = 4 if g == NGROUPS - 1 else 2
                for pp in range(nparts):
                    lo, hi = pp * POOLW // nparts, (pp + 1) * POOLW // nparts
                    nc.sync.dma_start(out=p1_d[g, :, lo:hi], in_=p1[:, lo:hi])
    if not nc.is_finalized():
        nc.finalize()
    return nc


def _prep_inputs(x):
    x64 = x.astype(np.float64)
    x2 = (x64 * x64).sum(-1)
    neg_half_x2 = (-0.5 * x2).astype(np.float32)
    in_maps = []
    for c in range(NCORES):
        b, h = c // 2, c % 2
        q = np.zeros((KDIM, QROWS), np.float32)
        q[:D] = x[b, h * QROWS:(h + 1) * QROWS, :].T
        q[D] = 1.0
        kv = np.zeros((KDIM, M), np.float32)
        kv[:D] = x[b].T
        kv[D] = neg_half_x2[b]
        # permute into 4 tiles: tile d = cols [1024d:1024d+1024) ++ [4096+1024d:...)
        kvp = np.empty_like(kv)
        for d in range(NBLK):
            kvp[:, 2048 * d:2048 * d + 1024] = kv[:, 1024 * d:1024 * (d + 1)]
            kvp[:, 2048 * d + 1024:2048 * (d + 1)] = kv[:, POOLW + 1024 * d:POOLW + 1024 * (d + 1)]
        in_maps.append({"q": q, "kv": kvp})
    return in_maps


def _rerank_batch(xb, p1b):
    x2f = (xb * xb).sum(-1)
    p = p1b.astype(np.float32)
    np.nan_to_num(p, copy=False, nan=-np.inf)
    part = np.argpartition(-p, T_CAND - 1, axis=1)[:, :T_CAND]
    pv = np.take_along_axis(p, part, axis=1)
    pvs = -np.sort(-pv, axis=1)
    suspect = pvs[:, T_CAND - 1] >= pvs[:, K - 1] - DELTA
    cand = (part[:, :, None] + POOLW * np.arange(EXPAND)[None, None, :]
            ).reshape(M, T_CAND * EXPAND)
    idx = np.empty((M, K), np.int64)
    BS = 2048
    for s in range(0, M, BS):
        e = min(s + BS, M)
        cb = cand[s:e]
        g = xb[cb]
        dots = np.matmul(g, xb[s:e, :, None])[..., 0].astype(np.float32)
        d2c = x2f[s:e, None] + x2f[cb] - 2.0 * dots
        order = np.lexsort((cb, d2c), axis=1)[:, :K]
        idx[s:e] = np.take_along_axis(cb, order, axis=1)
    return idx, suspect


def _host_topk_row(xb, x2f, r):
    d2 = x2f + x2f[r] - 2.0 * (xb @ xb[r]).astype(np.float32)
    order = np.lexsort((np.arange(M), d2))[:K]
    return order


def kernel(x, k):
    x = np.asarray(x, dtype=np.float32)
    k = int(k)
    assert x.shape == (N, M, D) and k == K

    from concourse.bass_utils import run_bass_kernel_spmd

    if "nc" not in _COMPILED:
        _COMPILED["nc"] = _build_nc()
    nc = _COMPILED["nc"]

    in_maps = _prep_inputs(x)
    res = run_bass_kernel_spmd(nc, in_maps, list(range(NCORES))).results

    p1 = np.empty((N, M, POOLW), np.float16)
    for c in range(NCORES):
        b, h = c // 2, c % 2
        sl = slice(h * QROWS, (h + 1) * QROWS)
        p1[b, sl] = res[c]["p1"].reshape(QROWS, POOLW)

    idx = np.empty((N, M, K), np.int64)
    for b in range(N):
        idx_b, suspect = _rerank_batch(x[b], p1[b])
        idx[b] = idx_b
        rows = np.nonzero(suspect)[0]
        if rows.size:
            x2f = (x[b] * x[b]).sum(-1)
            for r in rows:
                idx[b, r] = _host_topk_row(x[b], x2f, r)

    offset = (np.arange(N, dtype=np.int64) * M)[:, None, None]
    src = (idx + offset).reshape(-1).astype(np.int32)
    dst = np.repeat(np.arange(N * M, dtype=np.int32), K)
    return src, dst


if __name__ == "__main__":
    rng = np.random.default_rng(0)
    xt = rng.standard_normal((N, M, D), dtype=np.float32)
    s, d = kernel(xt, 16)
    print(s[:32], d[:32])
